# revision 11
# baseline (speedup 1.0000x reference)
"""Multi-head attention kernel for 8 Trainium2 NeuronCores.

Problem: B=2, S=2048, D=1024, H=16 heads (Dh=64).
    qh = split(q @ wq.T + bq); kh, vh likewise
    out = concat_h(softmax(qh kh^T / 8) vh) @ wo.T + bo

Sharding: core c = 4*b + j handles batch b and head group j (4 heads,
channels [256j, 256j+256)).  Each core computes its 4 heads' attention and
a partial output projection; the host sums the 4 partials per batch and
adds bo.

On-chip layout (all matmul operands float32r = TF32-like, full PE rate):
  - activations are pre-transposed on the host: qT/kT/vT = x[b].T [1024,2048]
  - QT, KT [256chan, 2048tok] (chan on partitions), V [2048tok, 256chan]
  - scores computed transposed: S^T[k, q] = KT_h^T-slice . QT_h (row-packed
    head pairs), softmax-exp on ACT straight out of PSUM (no max subtraction:
    scores ~ N(0,1) after the 1/8 scale)
  - row sums via ones-matmul; P@V as C^T = V^T @ E^T (col-packed head pairs)
  - bv folded in via rank-1 update C += bv^T . sums before normalization
  - normalization: C_norm = C * broadcast(1/sums) (broadcast via K=1 matmul)
  - out_partial[tok, :] = C_norm^T-chunks . woT  (token-major, DMA'd out)
"""

import numpy as np
import concourse.bass as bass
import concourse.tile as tile
import concourse.mybir as mybir
from concourse import bacc, bass_utils

B, S, D, H = 2, 2048, 1024, 16
DH = 64
HL = 4            # heads per core
CL = HL * DH      # local channels = 256
N_CORES = 8

f32 = mybir.dt.float32
f32r = mybir.dt.float32r
AF = mybir.ActivationFunctionType

TB = 4            # token blocks for projections (512 tokens each)
TBS = S // TB     # 512
QB = 4            # query blocks for attention (512 queries each)
QBS = S // QB     # 512
KT_N = S // 128   # 16 key tiles


def build():
    nc = bacc.Bacc("TRN2", debug=False, num_devices=N_CORES)
    qT = nc.dram_tensor("qT", [D, S], f32r, kind="ExternalInput").ap()
    kT = nc.dram_tensor("kT", [D, S], f32r, kind="ExternalInput").ap()
    vT = nc.dram_tensor("vT", [D, S], f32r, kind="ExternalInput").ap()
    wqT = nc.dram_tensor("wqT", [D, CL], f32r, kind="ExternalInput").ap()
    wkT = nc.dram_tensor("wkT", [D, CL], f32r, kind="ExternalInput").ap()
    wvT = nc.dram_tensor("wvT", [D, CL], f32r, kind="ExternalInput").ap()
    woT = nc.dram_tensor("woT", [CL, D], f32r, kind="ExternalInput").ap()
    bq = nc.dram_tensor("bq", [CL], f32, kind="ExternalInput").ap()
    bk = nc.dram_tensor("bk", [CL], f32, kind="ExternalInput").ap()
    vones = nc.dram_tensor("vones", [128, KT_N], f32r, kind="ExternalInput").ap()
    bvz = nc.dram_tensor("bvz", [2 * (CL // 2 + 2)], f32r, kind="ExternalInput").ap()
    out = nc.dram_tensor("out", [S, D], f32, kind="ExternalOutput").ap()

    with tile.TileContext(nc) as tc:
        with (
            tc.tile_pool(name="wp", bufs=1) as wp,
            tc.tile_pool(name="xp", bufs=2) as xp,
            tc.tile_pool(name="qkv", bufs=1) as qkv,
            tc.tile_pool(name="cp", bufs=1) as cp,
            tc.tile_pool(name="ep", bufs=3) as ep,
            tc.tile_pool(name="rp", bufs=2) as rp,
            tc.tile_pool(name="op", bufs=2) as op,
            tc.tile_pool(name="pp", bufs=2, space="PSUM") as pp,
            tc.tile_pool(name="sp", bufs=2, space="PSUM") as sp,
            tc.tile_pool(name="cps", bufs=1, space="PSUM") as cps,
        ):
            # ---- weights / constants (resident) ----
            wq_sb = wp.tile([128, 8, CL], f32r)
            wk_sb = wp.tile([128, 8, CL], f32r)
            wv_sb = wp.tile([128, 8, CL], f32r)
            wo_sb = wp.tile([128, 2, D], f32r)
            for c in range(8):
                nc.sync.dma_start(wq_sb[:, c], wqT[c * 128:(c + 1) * 128, :])
                nc.sync.dma_start(wk_sb[:, c], wkT[c * 128:(c + 1) * 128, :])
                nc.sync.dma_start(wv_sb[:, c], wvT[c * 128:(c + 1) * 128, :])
            for c in range(2):
                nc.sync.dma_start(wo_sb[:, c], woT[c * 128:(c + 1) * 128, :])
            bq_sb = wp.tile([128, 2], f32)
            bk_sb = wp.tile([128, 2], f32)
            nc.sync.dma_start(bq_sb, bq.rearrange("(c p) -> p c", p=128))
            nc.sync.dma_start(bk_sb, bk.rearrange("(c p) -> p c", p=128))
            # [0, bv_a, 0, bv_b] per head pair (lhsT of rank-1 update)
            bvz_sb = wp.tile([65, 260], f32r)
            nc.sync.dma_start(bvz_sb[0:1, :], bvz[None, :])
            nc.sync.dma_start(bvz_sb[64:65, :], bvz[None, :])
            ones_f32 = wp.tile([128, 64], f32)
            nc.vector.memset(ones_f32, 1.0)
            ones_sb = wp.tile([128, 64], f32r)
            nc.vector.tensor_copy(ones_sb, ones_f32)

            # ---- projections ----
            QT = qkv.tile([128, 2, S], f32r)   # [chan, tok]
            KT = qkv.tile([128, 2, S], f32r)
            V = qkv.tile([128, KT_N, 260], f32r)  # [V_a|ones|V_b|ones] x 2 pairs
            for col in (64, 129, 194, 259):
                nc.sync.dma_start(V[:, :, col], vones)

            for tb in range(TB):
                t0 = tb * TBS
                xq = xp.tile([128, 8, TBS], f32r, tag="x")
                for c in range(8):
                    nc.sync.dma_start(xq[:, c], qT[c * 128:(c + 1) * 128, t0:t0 + TBS])
                for cc in range(2):
                    ps = pp.tile([128, TBS], f32, tag="pp")
                    for c in range(8):
                        nc.tensor.matmul(ps, wq_sb[:, c, cc * 128:(cc + 1) * 128],
                                         xq[:, c], start=(c == 0), stop=(c == 7))
                    nc.vector.tensor_scalar_add(QT[:, cc, t0:t0 + TBS], ps,
                                                bq_sb[:, cc:cc + 1])
                xk = xp.tile([128, 8, TBS], f32r, tag="x")
                for c in range(8):
                    nc.sync.dma_start(xk[:, c], kT[c * 128:(c + 1) * 128, t0:t0 + TBS])
                for cc in range(2):
                    ps = pp.tile([128, TBS], f32, tag="pp")
                    for c in range(8):
                        nc.tensor.matmul(ps, wk_sb[:, c, cc * 128:(cc + 1) * 128],
                                         xk[:, c], start=(c == 0), stop=(c == 7))
                    nc.vector.tensor_scalar_add(KT[:, cc, t0:t0 + TBS], ps,
                                                bk_sb[:, cc:cc + 1])
                xv = xp.tile([128, 8, TBS], f32r, tag="x")
                for c in range(8):
                    nc.sync.dma_start(xv[:, c], vT[c * 128:(c + 1) * 128, t0:t0 + TBS])
                for tt in range(TBS // 128):
                    ps = pp.tile([128, CL], f32, tag="pp")
                    for c in range(8):
                        nc.tensor.matmul(ps, xv[:, c, tt * 128:(tt + 1) * 128],
                                         wv_sb[:, c], start=(c == 0), stop=(c == 7))
                    T = tb * 4 + tt
                    nc.vector.tensor_copy(V[:, T, 0:64], ps[:, 0:64])
                    nc.vector.tensor_copy(V[:, T, 65:129], ps[:, 64:128])
                    nc.vector.tensor_copy(V[:, T, 130:194], ps[:, 128:192])
                    nc.vector.tensor_copy(V[:, T, 195:259], ps[:, 192:256])

            # ---- attention + output projection, per query block ----
            C = cp.tile([128, 2, S], f32r)   # C^T [cat-chan, tok]
            for qb in range(QB):
                q0 = qb * QBS
                for hp in range(2):
                    c_a = cps.tile([65, QBS], f32, tag="ca", name="c_a")
                    c_b = cps.tile([65, QBS], f32, tag="cb", name="c_b")
                    for kt in range(KT_N):
                        k0 = kt * 128
                        s_ps = sp.tile([128, 2 * QBS], f32, tag="s")
                        # transposed scores for the head pair (row-packed)
                        nc.tensor.matmul(s_ps[:, 0:QBS],
                                         KT[0:64, hp, k0:k0 + 128],
                                         QT[0:64, hp, q0:q0 + QBS])
                        nc.tensor.matmul(s_ps[:, QBS:2 * QBS],
                                         KT[64:128, hp, k0:k0 + 128],
                                         QT[64:128, hp, q0:q0 + QBS])
                        e = ep.tile([128, 2 * QBS], f32r, tag="e")
                        nc.scalar.activation(e, s_ps, AF.Exp, scale=0.125)
                        # C^T accumulation; row 0 = softmax denominators (ones col)
                        nc.tensor.matmul(c_a, V[:, kt, 130 * hp:130 * hp + 65],
                                         e[:, 0:QBS], start=(kt == 0), stop=False)
                        nc.tensor.matmul(c_b, V[:, kt, 130 * hp + 65:130 * hp + 130],
                                         e[:, QBS:2 * QBS], start=(kt == 0), stop=False)
                    # rank-1 bv fold: C += [bv;0]^T . sums  ((C + bv*s)/s = C/s + bv)
                    s_a = rp.tile([65, QBS], f32r, tag="sa", name="s_a")
                    s_b = rp.tile([65, QBS], f32r, tag="sb", name="s_b")
                    nc.vector.tensor_copy(s_a[64:65, :], c_a[64:65, :])
                    nc.vector.tensor_copy(s_b[64:65, :], c_b[64:65, :])
                    nc.tensor.matmul(c_a, bvz_sb[64:65, 130 * hp:130 * hp + 65],
                                     s_a[64:65, :], start=False, stop=True)
                    nc.tensor.matmul(c_b, bvz_sb[64:65, 130 * hp + 65:130 * hp + 130],
                                     s_b[64:65, :], start=False, stop=True)
                    # reciprocal + broadcast to 64 partitions
                    r_a = rp.tile([65, QBS], f32r, tag="ra", name="r_a")
                    r_b = rp.tile([65, QBS], f32r, tag="rb", name="r_b")
                    with nc.allow_low_precision(reason="softmax norm in f32r"):
                        nc.vector.reciprocal(r_a[64:65, :], s_a[64:65, :])
                        nc.vector.reciprocal(r_b[64:65, :], s_b[64:65, :])
                    b_sb = rp.tile([128, QBS], f32r, tag="b")
                    b_ps = pp.tile([64, QBS], f32, tag="pp", name="b_ps")
                    nc.tensor.matmul(b_ps, ones_sb[64:65, :], r_a[64:65, :])
                    nc.vector.tensor_copy(b_sb[0:64, :], b_ps)
                    b_ps2 = pp.tile([64, QBS], f32, tag="pp", name="b_ps2")
                    nc.tensor.matmul(b_ps2, ones_sb[64:65, :], r_b[64:65, :])
                    nc.vector.tensor_copy(b_sb[64:128, :], b_ps2)
                    nc.vector.tensor_mul(C[0:64, hp, q0:q0 + QBS],
                                         c_a[0:64, :], b_sb[0:64, :])
                    nc.vector.tensor_mul(C[64:128, hp, q0:q0 + QBS],
                                         c_b[0:64, :], b_sb[64:128, :])
                # output projection for this query block's tokens
                for tt in range(QBS // 128):
                    tg = q0 + tt * 128
                    o = op.tile([128, D], f32, tag="o")
                    for nb in range(2):
                        ps = pp.tile([128, 512], f32, tag="pp")
                        for cc in range(2):
                            nc.tensor.matmul(ps, C[:, cc, tg:tg + 128],
                                             wo_sb[:, cc, nb * 512:(nb + 1) * 512],
                                             start=(cc == 0), stop=(cc == 1))
                        nc.vector.tensor_copy(o[:, nb * 512:(nb + 1) * 512], ps)
                    nc.sync.dma_start(out[tg:tg + 128, :], o)

    nc.compile()
    return nc


_CACHE = {}


def _get_nc():
    if "nc" not in _CACHE:
        _CACHE["nc"] = build()
    return _CACHE["nc"]


def make_in_maps(q, k, v, wq, bq, wk, bk, wv, bv, wo, bo):
    xT = {}
    for b in range(B):
        xT[("q", b)] = np.ascontiguousarray(q[b].T, dtype=np.float32)
        xT[("k", b)] = np.ascontiguousarray(k[b].T, dtype=np.float32)
        xT[("v", b)] = np.ascontiguousarray(v[b].T, dtype=np.float32)
    in_maps = []
    for core in range(N_CORES):
        b, j = divmod(core, N_CORES // B)
        sl = slice(CL * j, CL * (j + 1))
        bv_loc = np.asarray(bv[sl], dtype=np.float32)
        bvz_local = np.zeros(260, dtype=np.float32)
        for hp in range(2):
            bvz_local[130 * hp + 0:130 * hp + 64] = bv_loc[128 * hp:128 * hp + 64]
            bvz_local[130 * hp + 65:130 * hp + 129] = bv_loc[128 * hp + 64:128 * hp + 128]
        in_maps.append({
            "qT": xT[("q", b)],
            "kT": xT[("k", b)],
            "vT": xT[("v", b)],
            "wqT": np.ascontiguousarray(wq[sl, :].T, dtype=np.float32),
            "wkT": np.ascontiguousarray(wk[sl, :].T, dtype=np.float32),
            "wvT": np.ascontiguousarray(wv[sl, :].T, dtype=np.float32),
            "woT": np.ascontiguousarray(wo[:, sl].T, dtype=np.float32),
            "bq": np.ascontiguousarray(bq[sl], dtype=np.float32),
            "bk": np.ascontiguousarray(bk[sl], dtype=np.float32),
            "vones": np.ones((128, KT_N), dtype=np.float32),
            "bvz": bvz_local,
        })
    return in_maps


def combine(results, bo):
    GP = N_CORES // B
    out = np.empty((B, S, D), dtype=np.float32)
    for b in range(B):
        acc = results[b * GP]["out"].astype(np.float32).copy()
        for j in range(1, GP):
            acc += results[b * GP + j]["out"]
        out[b] = acc + bo[None, :].astype(np.float32)
    return out


def kernel(q, k, v, wq, bq, wk, bk, wv, bv, wo, bo):
    nc = _get_nc()
    in_maps = make_in_maps(q, k, v, wq, bq, wk, bk, wv, bv, wo, bo)
    res = bass_utils.run_bass_kernel_spmd(nc, in_maps, core_ids=list(range(N_CORES)))
    return combine(res.results, bo)


# revision 13
# speedup vs baseline: 1.3148x; 1.3148x over previous
"""Multi-head attention kernel for 8 Trainium2 NeuronCores.

Problem: B=2, S=2048, D=1024, H=16 heads (Dh=64).
    qh = split(q @ wq.T + bq); kh, vh likewise
    out = concat_h(softmax(qh kh^T / 8) vh) @ wo.T + bo

Sharding: core c = 4*b + j handles batch b and head group j (4 heads,
channels [256j, 256j+256)).  Each core computes its 4 heads' attention and
a partial output projection; the host sums the 4 partials per batch and
adds bo.

All matmuls run in bf16 (1 cycle/row on the PE) with fp32 PSUM
accumulation; the softmax denominator path stays fp32.  Host pre-casts
inputs/weights to bf16 and pre-transposes activations so every operand
lands with its contraction dim on partitions:
  - QT, KT [256chan, 2048tok] (chan on partitions), V [2048tok, 256chan]
  - scores computed transposed: S^T[k, q] = KT_h-slice^T . QT_h (row-packed
    head pairs), exp on ACT straight out of PSUM (no max subtraction:
    scores ~ N(0,1) after the 1/8 scale)
  - V carries a ones column per head -> row 64 of the C^T accumulation is
    the softmax denominator (M=65 matmuls, PSUM base 0 only)
  - bv folded in via rank-1 update C += [bv;0]^T . sums pre-normalization
  - normalization: broadcast sums via K=1 matmul, reciprocal on ACT,
    single DVE multiply per head
  - out_partial[tok, :] = C_norm^T-chunks . woT  (token-major, DMA'd out)
"""

import numpy as np
import ml_dtypes
import concourse.bass as bass
import concourse.tile as tile
import concourse.mybir as mybir
from concourse import bacc, bass_utils

B, S, D, H = 2, 2048, 1024, 16
DH = 64
HL = 4            # heads per core
CL = HL * DH      # local channels = 256
N_CORES = 8

f32 = mybir.dt.float32
bf16 = mybir.dt.bfloat16
AF = mybir.ActivationFunctionType
BF = ml_dtypes.bfloat16

TB = 4            # token blocks for projections (512 tokens each)
TBS = S // TB     # 512
QB = 4            # query blocks for attention (512 queries each)
QBS = S // QB     # 512
KT_N = S // 128   # 16 key tiles


def build():
    nc = bacc.Bacc("TRN2", debug=False, num_devices=N_CORES)
    qT = nc.dram_tensor("qT", [D, S], bf16, kind="ExternalInput").ap()
    kT = nc.dram_tensor("kT", [D, S], bf16, kind="ExternalInput").ap()
    vT = nc.dram_tensor("vT", [D, S], bf16, kind="ExternalInput").ap()
    wqT = nc.dram_tensor("wqT", [D, CL], bf16, kind="ExternalInput").ap()
    wkT = nc.dram_tensor("wkT", [D, CL], bf16, kind="ExternalInput").ap()
    wvT = nc.dram_tensor("wvT", [D, CL], bf16, kind="ExternalInput").ap()
    woT = nc.dram_tensor("woT", [CL, D], bf16, kind="ExternalInput").ap()
    bq = nc.dram_tensor("bq", [CL], f32, kind="ExternalInput").ap()
    bk = nc.dram_tensor("bk", [CL], f32, kind="ExternalInput").ap()
    vones = nc.dram_tensor("vones", [128, KT_N], bf16, kind="ExternalInput").ap()
    bvz = nc.dram_tensor("bvz", [260], bf16, kind="ExternalInput").ap()
    out = nc.dram_tensor("out", [S, D], f32, kind="ExternalOutput").ap()

    with tile.TileContext(nc) as tc:
        with (
            tc.tile_pool(name="wp", bufs=1) as wp,
            tc.tile_pool(name="xp", bufs=3) as xp,
            tc.tile_pool(name="qkv", bufs=1) as qkv,
            tc.tile_pool(name="cp", bufs=1) as cp,
            tc.tile_pool(name="ep", bufs=3) as ep,
            tc.tile_pool(name="rp", bufs=2) as rp,
            tc.tile_pool(name="op", bufs=2) as op,
            tc.tile_pool(name="pp", bufs=2, space="PSUM") as pp,
            tc.tile_pool(name="sp", bufs=2, space="PSUM") as sp,
            tc.tile_pool(name="cps", bufs=1, space="PSUM") as cps,
        ):
            # ---- weights / constants (resident) ----
            wq_sb = wp.tile([128, 8, CL], bf16)
            wk_sb = wp.tile([128, 8, CL], bf16)
            wv_sb = wp.tile([128, 8, CL], bf16)
            wo_sb = wp.tile([128, 2, D], bf16)
            for c in range(8):
                nc.sync.dma_start(wq_sb[:, c], wqT[c * 128:(c + 1) * 128, :])
                nc.sync.dma_start(wk_sb[:, c], wkT[c * 128:(c + 1) * 128, :])
                nc.sync.dma_start(wv_sb[:, c], wvT[c * 128:(c + 1) * 128, :])
            for c in range(2):
                nc.sync.dma_start(wo_sb[:, c], woT[c * 128:(c + 1) * 128, :])
            bq_sb = wp.tile([128, 2], f32)
            bk_sb = wp.tile([128, 2], f32)
            nc.sync.dma_start(bq_sb, bq.rearrange("(c p) -> p c", p=128))
            nc.sync.dma_start(bk_sb, bk.rearrange("(c p) -> p c", p=128))
            # [bv_a, 0, bv_b, 0] per head pair (lhsT of rank-1 update)
            bvz_sb = wp.tile([65, 260], bf16)
            nc.sync.dma_start(bvz_sb[64:65, :], bvz[None, :])
            ones_sb = wp.tile([128, 64], bf16)
            nc.vector.memset(ones_sb, 1.0)

            # ---- projections ----
            QT = qkv.tile([128, 2, S], bf16)   # [chan, tok]
            KT = qkv.tile([128, 2, S], bf16)
            V = qkv.tile([128, KT_N, 260], bf16)  # [V_a|ones|V_b|ones] x 2 pairs
            for col in (64, 129, 194, 259):
                nc.sync.dma_start(V[:, :, col], vones)

            for tb in range(TB):
                t0 = tb * TBS
                xq = xp.tile([128, 8, TBS], bf16, tag="x")
                for c in range(8):
                    nc.sync.dma_start(xq[:, c], qT[c * 128:(c + 1) * 128, t0:t0 + TBS])
                for cc in range(2):
                    ps = pp.tile([128, TBS], f32, tag="pp")
                    for c in range(8):
                        nc.tensor.matmul(ps, wq_sb[:, c, cc * 128:(cc + 1) * 128],
                                         xq[:, c], start=(c == 0), stop=(c == 7))
                    nc.vector.tensor_scalar_add(QT[:, cc, t0:t0 + TBS], ps,
                                                bq_sb[:, cc:cc + 1])
                xk = xp.tile([128, 8, TBS], bf16, tag="x")
                for c in range(8):
                    nc.sync.dma_start(xk[:, c], kT[c * 128:(c + 1) * 128, t0:t0 + TBS])
                for cc in range(2):
                    ps = pp.tile([128, TBS], f32, tag="pp")
                    for c in range(8):
                        nc.tensor.matmul(ps, wk_sb[:, c, cc * 128:(cc + 1) * 128],
                                         xk[:, c], start=(c == 0), stop=(c == 7))
                    nc.vector.tensor_scalar_add(KT[:, cc, t0:t0 + TBS], ps,
                                                bk_sb[:, cc:cc + 1])
                xv = xp.tile([128, 8, TBS], bf16, tag="x")
                for c in range(8):
                    nc.sync.dma_start(xv[:, c], vT[c * 128:(c + 1) * 128, t0:t0 + TBS])
                for tt in range(TBS // 128):
                    ps = pp.tile([128, CL], f32, tag="pp")
                    for c in range(8):
                        nc.tensor.matmul(ps, xv[:, c, tt * 128:(tt + 1) * 128],
                                         wv_sb[:, c], start=(c == 0), stop=(c == 7))
                    T = tb * 4 + tt
                    nc.vector.tensor_copy(V[:, T, 0:64], ps[:, 0:64])
                    nc.vector.tensor_copy(V[:, T, 65:129], ps[:, 64:128])
                    nc.vector.tensor_copy(V[:, T, 130:194], ps[:, 128:192])
                    nc.vector.tensor_copy(V[:, T, 195:259], ps[:, 192:256])

            # ---- attention + output projection, per query block ----
            C = cp.tile([128, 2, S], bf16)   # C^T [cat-chan, tok]
            for qb in range(QB):
                q0 = qb * QBS
                for hp in range(2):
                    c_a = cps.tile([65, QBS], f32, tag="ca", name="c_a")
                    c_b = cps.tile([65, QBS], f32, tag="cb", name="c_b")
                    for kt in range(KT_N):
                        k0 = kt * 128
                        s_ps = sp.tile([128, 2 * QBS], f32, tag="s")
                        # transposed scores for the head pair (row-packed)
                        nc.tensor.matmul(s_ps[:, 0:QBS],
                                         KT[0:64, hp, k0:k0 + 128],
                                         QT[0:64, hp, q0:q0 + QBS])
                        nc.tensor.matmul(s_ps[:, QBS:2 * QBS],
                                         KT[64:128, hp, k0:k0 + 128],
                                         QT[64:128, hp, q0:q0 + QBS])
                        e = ep.tile([128, 2 * QBS], bf16, tag="e")
                        nc.scalar.activation(e, s_ps, AF.Exp, scale=0.125)
                        # C^T accumulation; row 64 = softmax denominators
                        nc.tensor.matmul(c_a, V[:, kt, 130 * hp:130 * hp + 65],
                                         e[:, 0:QBS], start=(kt == 0), stop=False)
                        nc.tensor.matmul(c_b, V[:, kt, 130 * hp + 65:130 * hp + 130],
                                         e[:, QBS:2 * QBS], start=(kt == 0), stop=False)
                    # rank-1 bv fold: C += [bv;0]^T . sums ((C + bv*s)/s = C/s + bv)
                    s_a = rp.tile([65, QBS], bf16, tag="sa", name="s_a")
                    s_b = rp.tile([65, QBS], bf16, tag="sb", name="s_b")
                    nc.vector.tensor_copy(s_a[64:65, :], c_a[64:65, :])
                    nc.vector.tensor_copy(s_b[64:65, :], c_b[64:65, :])
                    nc.tensor.matmul(c_a, bvz_sb[64:65, 130 * hp:130 * hp + 65],
                                     s_a[64:65, :], start=False, stop=True)
                    nc.tensor.matmul(c_b, bvz_sb[64:65, 130 * hp + 65:130 * hp + 130],
                                     s_b[64:65, :], start=False, stop=True)
                    # broadcast sums to 64 partitions, reciprocal on ACT
                    b_ps = pp.tile([64, QBS], f32, tag="pp", name="b_ps")
                    nc.tensor.matmul(b_ps, ones_sb[64:65, :], s_a[64:65, :])
                    r_a = rp.tile([64, QBS], f32, tag="ra", name="r_a")
                    nc.vector.reciprocal_approx_fast(r_a, b_ps)
                    b_ps2 = pp.tile([64, QBS], f32, tag="pp", name="b_ps2")
                    nc.tensor.matmul(b_ps2, ones_sb[64:65, :], s_b[64:65, :])
                    r_b = rp.tile([64, QBS], f32, tag="rb", name="r_b")
                    nc.vector.reciprocal_approx_fast(r_b, b_ps2)
                    nc.vector.tensor_mul(C[0:64, hp, q0:q0 + QBS], c_a[0:64, :], r_a)
                    nc.vector.tensor_mul(C[64:128, hp, q0:q0 + QBS], c_b[0:64, :], r_b)
                # output projection for this query block's tokens
                for tt in range(QBS // 128):
                    tg = q0 + tt * 128
                    o = op.tile([128, D], f32, tag="o")
                    for nb in range(2):
                        ps = pp.tile([128, 512], f32, tag="pp")
                        for cc in range(2):
                            nc.tensor.matmul(ps, C[:, cc, tg:tg + 128],
                                             wo_sb[:, cc, nb * 512:(nb + 1) * 512],
                                             start=(cc == 0), stop=(cc == 1))
                        nc.vector.tensor_copy(o[:, nb * 512:(nb + 1) * 512], ps)
                    nc.sync.dma_start(out[tg:tg + 128, :], o)

    nc.compile()
    return nc


_CACHE = {}


def _get_nc():
    if "nc" not in _CACHE:
        _CACHE["nc"] = build()
    return _CACHE["nc"]


def make_in_maps(q, k, v, wq, bq, wk, bk, wv, bv, wo, bo):
    xT = {}
    for b in range(B):
        xT[("q", b)] = np.ascontiguousarray(np.asarray(q[b]).T).astype(BF)
        xT[("k", b)] = np.ascontiguousarray(np.asarray(k[b]).T).astype(BF)
        xT[("v", b)] = np.ascontiguousarray(np.asarray(v[b]).T).astype(BF)
    in_maps = []
    for core in range(N_CORES):
        b, j = divmod(core, N_CORES // B)
        sl = slice(CL * j, CL * (j + 1))
        bv_loc = np.asarray(bv[sl], dtype=np.float32)
        bvz_local = np.zeros(260, dtype=np.float32)
        for hp in range(2):
            bvz_local[130 * hp + 0:130 * hp + 64] = bv_loc[128 * hp:128 * hp + 64]
            bvz_local[130 * hp + 65:130 * hp + 129] = bv_loc[128 * hp + 64:128 * hp + 128]
        in_maps.append({
            "qT": xT[("q", b)],
            "kT": xT[("k", b)],
            "vT": xT[("v", b)],
            "wqT": np.ascontiguousarray(np.asarray(wq)[sl, :].T).astype(BF),
            "wkT": np.ascontiguousarray(np.asarray(wk)[sl, :].T).astype(BF),
            "wvT": np.ascontiguousarray(np.asarray(wv)[sl, :].T).astype(BF),
            "woT": np.ascontiguousarray(np.asarray(wo)[:, sl].T).astype(BF),
            "bq": np.ascontiguousarray(bq[sl], dtype=np.float32),
            "bk": np.ascontiguousarray(bk[sl], dtype=np.float32),
            "vones": np.ones((128, KT_N), dtype=BF),
            "bvz": bvz_local.astype(BF),
        })
    return in_maps


def combine(results, bo):
    GP = N_CORES // B
    out = np.empty((B, S, D), dtype=np.float32)
    for b in range(B):
        acc = results[b * GP]["out"].astype(np.float32).copy()
        for j in range(1, GP):
            acc += results[b * GP + j]["out"]
        out[b] = acc + np.asarray(bo, dtype=np.float32)[None, :]
    return out


def kernel(q, k, v, wq, bq, wk, bk, wv, bv, wo, bo):
    nc = _get_nc()
    in_maps = make_in_maps(q, k, v, wq, bq, wk, bk, wv, bv, wo, bo)
    res = bass_utils.run_bass_kernel_spmd(nc, in_maps, core_ids=list(range(N_CORES)))
    return combine(res.results, bo)


# revision 14
# speedup vs baseline: 1.3692x; 1.0414x over previous
"""Multi-head attention kernel for 8 Trainium2 NeuronCores.

Problem: B=2, S=2048, D=1024, H=16 heads (Dh=64).
    qh = split(q @ wq.T + bq); kh, vh likewise
    out = concat_h(softmax(qh kh^T / 8) vh) @ wo.T + bo

Sharding: core c = 4*b + j handles batch b and head group j (4 heads,
channels [256j, 256j+256)).  Each core computes its 4 heads' attention and
a partial output projection; the host sums the 4 partials per batch and
adds bo.

All matmuls run in bf16 (1 cycle/row on the PE) with fp32 PSUM
accumulation; the softmax denominator path stays fp32.  Host pre-casts
inputs/weights to bf16 and pre-transposes activations so every operand
lands with its contraction dim on partitions:
  - QT, KT [256chan, 2048tok] (chan on partitions), V [2048tok, 256chan]
  - scores computed transposed: S^T[k, q] = KT_h-slice^T . QT_h (row-packed
    head pairs), exp on ACT straight out of PSUM (no max subtraction:
    scores ~ N(0,1) after the 1/8 scale)
  - V carries a ones column per head -> row 64 of the C^T accumulation is
    the softmax denominator (M=65 matmuls, PSUM base 0 only)
  - bv folded in via rank-1 update C += [bv;0]^T . sums pre-normalization
  - normalization: broadcast sums via K=1 matmul, reciprocal on ACT,
    single DVE multiply per head
  - out_partial[tok, :] = C_norm^T-chunks . woT  (token-major, DMA'd out)
"""

import numpy as np
import ml_dtypes
import concourse.bass as bass
import concourse.tile as tile
import concourse.mybir as mybir
from concourse import bacc, bass_utils

B, S, D, H = 2, 2048, 1024, 16
DH = 64
HL = 4            # heads per core
CL = HL * DH      # local channels = 256
N_CORES = 8

f32 = mybir.dt.float32
bf16 = mybir.dt.bfloat16
AF = mybir.ActivationFunctionType
BF = ml_dtypes.bfloat16

TB = 4            # token blocks for projections (512 tokens each)
TBS = S // TB     # 512
QB = 4            # query blocks for attention (512 queries each)
QBS = S // QB     # 512
KT_N = S // 128   # 16 key tiles


def build():
    nc = bacc.Bacc("TRN2", debug=False, num_devices=N_CORES)
    qT = nc.dram_tensor("qT", [D, S], bf16, kind="ExternalInput").ap()
    kT = nc.dram_tensor("kT", [D, S], bf16, kind="ExternalInput").ap()
    vT = nc.dram_tensor("vT", [D, S], bf16, kind="ExternalInput").ap()
    wqT = nc.dram_tensor("wqT", [D, CL], bf16, kind="ExternalInput").ap()
    wkT = nc.dram_tensor("wkT", [D, CL], bf16, kind="ExternalInput").ap()
    wvT = nc.dram_tensor("wvT", [D, CL], bf16, kind="ExternalInput").ap()
    woT = nc.dram_tensor("woT", [CL, D], bf16, kind="ExternalInput").ap()
    bq = nc.dram_tensor("bq", [CL], f32, kind="ExternalInput").ap()
    bk = nc.dram_tensor("bk", [CL], f32, kind="ExternalInput").ap()
    vones = nc.dram_tensor("vones", [128, KT_N], bf16, kind="ExternalInput").ap()
    out = nc.dram_tensor("out", [S, D], f32, kind="ExternalOutput").ap()

    with tile.TileContext(nc) as tc:
        with (
            tc.tile_pool(name="wp", bufs=1) as wp,
            tc.tile_pool(name="xp", bufs=3) as xp,
            tc.tile_pool(name="qkv", bufs=1) as qkv,
            tc.tile_pool(name="cp", bufs=1) as cp,
            tc.tile_pool(name="ep", bufs=3) as ep,
            tc.tile_pool(name="rp", bufs=2) as rp,
            tc.tile_pool(name="op", bufs=2) as op,
            tc.tile_pool(name="pp", bufs=2, space="PSUM") as pp,
            tc.tile_pool(name="sp", bufs=2, space="PSUM") as sp,
            tc.tile_pool(name="cps", bufs=1, space="PSUM") as cps,
        ):
            # ---- weights / constants (resident) ----
            wq_sb = wp.tile([128, 8, CL], bf16)
            wk_sb = wp.tile([128, 8, CL], bf16)
            wv_sb = wp.tile([128, 8, CL], bf16)
            wo_sb = wp.tile([128, 2, D], bf16)
            for c in range(8):
                nc.sync.dma_start(wq_sb[:, c], wqT[c * 128:(c + 1) * 128, :])
                nc.sync.dma_start(wk_sb[:, c], wkT[c * 128:(c + 1) * 128, :])
                nc.sync.dma_start(wv_sb[:, c], wvT[c * 128:(c + 1) * 128, :])
            for c in range(2):
                nc.sync.dma_start(wo_sb[:, c], woT[c * 128:(c + 1) * 128, :])
            bq_sb = wp.tile([128, 2], f32)
            bk_sb = wp.tile([128, 2], f32)
            nc.sync.dma_start(bq_sb, bq.rearrange("(c p) -> p c", p=128))
            nc.sync.dma_start(bk_sb, bk.rearrange("(c p) -> p c", p=128))
            ones_sb = wp.tile([128, 64], bf16)
            nc.vector.memset(ones_sb, 1.0)

            # ---- projections ----
            QT = qkv.tile([128, 2, S], bf16)   # [chan, tok]
            KT = qkv.tile([128, 2, S], bf16)
            V = qkv.tile([128, KT_N, 260], bf16)  # [V_a|ones|V_b|ones] x 2 pairs
            for col in (64, 129, 194, 259):
                nc.sync.dma_start(V[:, :, col], vones)

            for tb in range(TB):
                t0 = tb * TBS
                xk = xp.tile([128, 8, TBS], bf16, tag="x")
                for c in range(8):
                    nc.sync.dma_start(xk[:, c], kT[c * 128:(c + 1) * 128, t0:t0 + TBS])
                for cc in range(2):
                    ps = pp.tile([128, TBS], f32, tag="pp")
                    for c in range(8):
                        nc.tensor.matmul(ps, wk_sb[:, c, cc * 128:(cc + 1) * 128],
                                         xk[:, c], start=(c == 0), stop=(c == 7))
                    nc.vector.tensor_scalar_add(KT[:, cc, t0:t0 + TBS], ps,
                                                bk_sb[:, cc:cc + 1])
            for tb in range(TB):
                t0 = tb * TBS
                xv = xp.tile([128, 8, TBS], bf16, tag="x")
                for c in range(8):
                    nc.sync.dma_start(xv[:, c], vT[c * 128:(c + 1) * 128, t0:t0 + TBS])
                for tt in range(TBS // 128):
                    ps = pp.tile([128, CL], f32, tag="pp")
                    for c in range(8):
                        nc.tensor.matmul(ps, xv[:, c, tt * 128:(tt + 1) * 128],
                                         wv_sb[:, c], start=(c == 0), stop=(c == 7))
                    T = tb * 4 + tt
                    nc.vector.tensor_copy(V[:, T, 0:64], ps[:, 0:64])
                    nc.vector.tensor_copy(V[:, T, 65:129], ps[:, 64:128])
                    nc.vector.tensor_copy(V[:, T, 130:194], ps[:, 128:192])
                    nc.vector.tensor_copy(V[:, T, 195:259], ps[:, 192:256])
                xq = xp.tile([128, 8, TBS], bf16, tag="x")
                for c in range(8):
                    nc.sync.dma_start(xq[:, c], qT[c * 128:(c + 1) * 128, t0:t0 + TBS])
                for cc in range(2):
                    ps = pp.tile([128, TBS], f32, tag="pp")
                    for c in range(8):
                        nc.tensor.matmul(ps, wq_sb[:, c, cc * 128:(cc + 1) * 128],
                                         xq[:, c], start=(c == 0), stop=(c == 7))
                    nc.vector.tensor_scalar_add(QT[:, cc, t0:t0 + TBS], ps,
                                                bq_sb[:, cc:cc + 1])

            # ---- attention + output projection, per query block ----
            C = cp.tile([128, 2, S], bf16)   # C^T [cat-chan, tok]
            for qb in range(QB):
                q0 = qb * QBS
                for hp in range(2):
                    c_a = cps.tile([65, QBS], f32, tag="ca", name="c_a")
                    c_b = cps.tile([65, QBS], f32, tag="cb", name="c_b")
                    for kt in range(KT_N):
                        k0 = kt * 128
                        s_ps = sp.tile([128, 2 * QBS], f32, tag="s")
                        # transposed scores for the head pair (row-packed)
                        nc.tensor.matmul(s_ps[:, 0:QBS],
                                         KT[0:64, hp, k0:k0 + 128],
                                         QT[0:64, hp, q0:q0 + QBS])
                        nc.tensor.matmul(s_ps[:, QBS:2 * QBS],
                                         KT[64:128, hp, k0:k0 + 128],
                                         QT[64:128, hp, q0:q0 + QBS])
                        e = ep.tile([128, 2 * QBS], bf16, tag="e")
                        nc.scalar.activation(e, s_ps, AF.Exp, scale=0.125)
                        # C^T accumulation; row 64 = softmax denominators
                        nc.tensor.matmul(c_a, V[:, kt, 130 * hp:130 * hp + 65],
                                         e[:, 0:QBS], start=(kt == 0),
                                         stop=(kt == KT_N - 1))
                        nc.tensor.matmul(c_b, V[:, kt, 130 * hp + 65:130 * hp + 130],
                                         e[:, QBS:2 * QBS], start=(kt == 0),
                                         stop=(kt == KT_N - 1))
                    # bv's contribution is a constant vector added on the host
                    s_a = rp.tile([65, QBS], bf16, tag="sa", name="s_a")
                    s_b = rp.tile([65, QBS], bf16, tag="sb", name="s_b")
                    nc.vector.tensor_copy(s_a[64:65, :], c_a[64:65, :])
                    nc.vector.tensor_copy(s_b[64:65, :], c_b[64:65, :])
                    # broadcast sums to 64 partitions, reciprocal on ACT
                    b_ps = pp.tile([64, QBS], f32, tag="pp", name="b_ps")
                    nc.tensor.matmul(b_ps, ones_sb[64:65, :], s_a[64:65, :])
                    r_a = rp.tile([64, QBS], f32, tag="ra", name="r_a")
                    nc.vector.reciprocal_approx_fast(r_a, b_ps)
                    b_ps2 = pp.tile([64, QBS], f32, tag="pp", name="b_ps2")
                    nc.tensor.matmul(b_ps2, ones_sb[64:65, :], s_b[64:65, :])
                    r_b = rp.tile([64, QBS], f32, tag="rb", name="r_b")
                    nc.vector.reciprocal_approx_fast(r_b, b_ps2)
                    nc.vector.tensor_mul(C[0:64, hp, q0:q0 + QBS], c_a[0:64, :], r_a)
                    nc.vector.tensor_mul(C[64:128, hp, q0:q0 + QBS], c_b[0:64, :], r_b)
                # output projection for this query block's tokens
                for tt in range(QBS // 128):
                    tg = q0 + tt * 128
                    o = op.tile([128, D], f32, tag="o")
                    for nb in range(2):
                        ps = pp.tile([128, 512], f32, tag="pp")
                        for cc in range(2):
                            nc.tensor.matmul(ps, C[:, cc, tg:tg + 128],
                                             wo_sb[:, cc, nb * 512:(nb + 1) * 512],
                                             start=(cc == 0), stop=(cc == 1))
                        nc.vector.tensor_copy(o[:, nb * 512:(nb + 1) * 512], ps)
                    nc.sync.dma_start(out[tg:tg + 128, :], o)

    nc.compile()
    return nc


_CACHE = {}


def _get_nc():
    if "nc" not in _CACHE:
        _CACHE["nc"] = build()
    return _CACHE["nc"]


def make_in_maps(q, k, v, wq, bq, wk, bk, wv, bv, wo, bo):
    xT = {}
    for b in range(B):
        xT[("q", b)] = np.ascontiguousarray(np.asarray(q[b]).T).astype(BF)
        xT[("k", b)] = np.ascontiguousarray(np.asarray(k[b]).T).astype(BF)
        xT[("v", b)] = np.ascontiguousarray(np.asarray(v[b]).T).astype(BF)
    in_maps = []
    for core in range(N_CORES):
        b, j = divmod(core, N_CORES // B)
        sl = slice(CL * j, CL * (j + 1))
        in_maps.append({
            "qT": xT[("q", b)],
            "kT": xT[("k", b)],
            "vT": xT[("v", b)],
            "wqT": np.ascontiguousarray(np.asarray(wq)[sl, :].T).astype(BF),
            "wkT": np.ascontiguousarray(np.asarray(wk)[sl, :].T).astype(BF),
            "wvT": np.ascontiguousarray(np.asarray(wv)[sl, :].T).astype(BF),
            "woT": np.ascontiguousarray(np.asarray(wo)[:, sl].T).astype(BF),
            "bq": np.ascontiguousarray(bq[sl], dtype=np.float32),
            "bk": np.ascontiguousarray(bk[sl], dtype=np.float32),
            "vones": np.ones((128, KT_N), dtype=BF),
        })
    return in_maps


def combine(results, bv, wo, bo):
    GP = N_CORES // B
    const = (np.asarray(bv, dtype=np.float64) @ np.asarray(wo, dtype=np.float64).T
             + np.asarray(bo, dtype=np.float64)).astype(np.float32)
    out = np.empty((B, S, D), dtype=np.float32)
    for b in range(B):
        acc = results[b * GP]["out"].astype(np.float32).copy()
        for j in range(1, GP):
            acc += results[b * GP + j]["out"]
        out[b] = acc + const[None, :]
    return out


def kernel(q, k, v, wq, bq, wk, bk, wv, bv, wo, bo):
    nc = _get_nc()
    in_maps = make_in_maps(q, k, v, wq, bq, wk, bk, wv, bv, wo, bo)
    res = bass_utils.run_bass_kernel_spmd(nc, in_maps, core_ids=list(range(N_CORES)))
    return combine(res.results, bv, wo, bo)


# revision 15
# speedup vs baseline: 1.4353x; 1.0483x over previous
"""Multi-head attention kernel for 8 Trainium2 NeuronCores.

Problem: B=2, S=2048, D=1024, H=16 heads (Dh=64).
    qh = split(q @ wq.T + bq); kh, vh likewise
    out = concat_h(softmax(qh kh^T / 8) vh) @ wo.T + bo

Sharding: core c = 4*b + j handles batch b and head group j (4 heads,
channels [256j, 256j+256)).  Each core computes its 4 heads' attention and
a partial output projection; the host sums the 4 partials per batch and
adds bo.

All matmuls run in bf16 (1 cycle/row on the PE) with fp32 PSUM
accumulation; the softmax denominator path stays fp32.  Host pre-casts
inputs/weights to bf16 and pre-transposes activations so every operand
lands with its contraction dim on partitions:
  - QT, KT [256chan, 2048tok] (chan on partitions), V [2048tok, 256chan]
  - scores computed transposed: S^T[k, q] = KT_h-slice^T . QT_h (row-packed
    head pairs), exp on ACT straight out of PSUM (no max subtraction:
    scores ~ N(0,1) after the 1/8 scale)
  - V carries a ones column per head -> row 64 of the C^T accumulation is
    the softmax denominator (M=65 matmuls, PSUM base 0 only)
  - bv folded in via rank-1 update C += [bv;0]^T . sums pre-normalization
  - normalization: broadcast sums via K=1 matmul, reciprocal on ACT,
    single DVE multiply per head
  - out_partial[tok, :] = C_norm^T-chunks . woT  (token-major, DMA'd out)
"""

import numpy as np
import ml_dtypes
import concourse.bass as bass
import concourse.tile as tile
import concourse.mybir as mybir
from concourse import bacc, bass_utils

B, S, D, H = 2, 2048, 1024, 16
DH = 64
HL = 4            # heads per core
CL = HL * DH      # local channels = 256
N_CORES = 8

f32 = mybir.dt.float32
bf16 = mybir.dt.bfloat16
AF = mybir.ActivationFunctionType
BF = ml_dtypes.bfloat16

TB = 4            # token blocks for projections (512 tokens each)
TBS = S // TB     # 512
QB = 4            # query blocks for attention (512 queries each)
QBS = S // QB     # 512
KT_N = S // 128   # 16 key tiles


def build():
    nc = bacc.Bacc("TRN2", debug=False, num_devices=N_CORES)
    qT = nc.dram_tensor("qT", [D, S], bf16, kind="ExternalInput").ap()
    kT = nc.dram_tensor("kT", [D, S], bf16, kind="ExternalInput").ap()
    vT = nc.dram_tensor("vT", [D, S], bf16, kind="ExternalInput").ap()
    wqT = nc.dram_tensor("wqT", [D, CL], bf16, kind="ExternalInput").ap()
    wkT = nc.dram_tensor("wkT", [D, CL], bf16, kind="ExternalInput").ap()
    wvT = nc.dram_tensor("wvT", [D, CL], bf16, kind="ExternalInput").ap()
    woT = nc.dram_tensor("woT", [CL, D], bf16, kind="ExternalInput").ap()
    bq = nc.dram_tensor("bq", [CL], f32, kind="ExternalInput").ap()
    bk = nc.dram_tensor("bk", [CL], f32, kind="ExternalInput").ap()
    vones = nc.dram_tensor("vones", [128, KT_N], bf16, kind="ExternalInput").ap()
    out = nc.dram_tensor("out", [S, D], f32, kind="ExternalOutput").ap()

    with tile.TileContext(nc) as tc:
        with (
            tc.tile_pool(name="wp", bufs=1) as wp,
            tc.tile_pool(name="xp", bufs=3) as xp,
            tc.tile_pool(name="qkv", bufs=1) as qkv,
            tc.tile_pool(name="cp", bufs=1) as cp,
            tc.tile_pool(name="ep", bufs=3) as ep,
            tc.tile_pool(name="rp", bufs=2) as rp,
            tc.tile_pool(name="op", bufs=2) as op,
            tc.tile_pool(name="pp", bufs=2, space="PSUM") as pp,
            tc.tile_pool(name="sp", bufs=2, space="PSUM") as sp,
            tc.tile_pool(name="cps", bufs=1, space="PSUM") as cps,
        ):
            # ---- weights / constants (resident) ----
            wq_sb = wp.tile([128, 8, CL], bf16)
            wk_sb = wp.tile([128, 8, CL], bf16)
            wv_sb = wp.tile([128, 8, CL], bf16)
            wo_sb = wp.tile([128, 2, D], bf16)
            for c in range(8):
                nc.sync.dma_start(wq_sb[:, c], wqT[c * 128:(c + 1) * 128, :])
                nc.sync.dma_start(wk_sb[:, c], wkT[c * 128:(c + 1) * 128, :])
                nc.sync.dma_start(wv_sb[:, c], wvT[c * 128:(c + 1) * 128, :])
            for c in range(2):
                nc.sync.dma_start(wo_sb[:, c], woT[c * 128:(c + 1) * 128, :])
            bq_sb = wp.tile([128, 2], f32)
            bk_sb = wp.tile([128, 2], f32)
            nc.sync.dma_start(bq_sb, bq.rearrange("(c p) -> p c", p=128))
            nc.sync.dma_start(bk_sb, bk.rearrange("(c p) -> p c", p=128))
            ones_sb = wp.tile([128, 64], bf16)
            nc.vector.memset(ones_sb, 1.0)

            # ---- projections ----
            QT = qkv.tile([128, 2, S], bf16)   # [chan, tok]
            KT = qkv.tile([128, 2, S], bf16)
            V = qkv.tile([128, KT_N, 260], bf16)  # [V_a|ones|V_b|ones] x 2 pairs
            for col in (64, 129, 194, 259):
                nc.sync.dma_start(V[:, :, col], vones)

            for tb in range(TB):
                t0 = tb * TBS
                xk = xp.tile([128, 8, TBS], bf16, tag="x")
                for c in range(8):
                    nc.sync.dma_start(xk[:, c], kT[c * 128:(c + 1) * 128, t0:t0 + TBS])
                for cc in range(2):
                    ps = pp.tile([128, TBS], f32, tag="pp")
                    for c in range(8):
                        nc.tensor.matmul(ps, wk_sb[:, c, cc * 128:(cc + 1) * 128],
                                         xk[:, c], start=(c == 0), stop=(c == 7))
                    nc.vector.tensor_scalar_add(KT[:, cc, t0:t0 + TBS], ps,
                                                bk_sb[:, cc:cc + 1])
            for tb in range(TB):
                t0 = tb * TBS
                xv = xp.tile([128, 8, TBS], bf16, tag="x")
                for c in range(8):
                    nc.sync.dma_start(xv[:, c], vT[c * 128:(c + 1) * 128, t0:t0 + TBS])
                for tt in range(TBS // 128):
                    ps = pp.tile([128, CL], f32, tag="pp")
                    for c in range(8):
                        nc.tensor.matmul(ps, xv[:, c, tt * 128:(tt + 1) * 128],
                                         wv_sb[:, c], start=(c == 0), stop=(c == 7))
                    T = tb * 4 + tt
                    nc.vector.tensor_copy(V[:, T, 0:64], ps[:, 0:64])
                    nc.vector.tensor_copy(V[:, T, 65:129], ps[:, 64:128])
                    nc.vector.tensor_copy(V[:, T, 130:194], ps[:, 128:192])
                    nc.vector.tensor_copy(V[:, T, 195:259], ps[:, 192:256])
                xq = xp.tile([128, 8, TBS], bf16, tag="x")
                for c in range(8):
                    nc.sync.dma_start(xq[:, c], qT[c * 128:(c + 1) * 128, t0:t0 + TBS])
                for cc in range(2):
                    ps = pp.tile([128, TBS], f32, tag="pp")
                    for c in range(8):
                        nc.tensor.matmul(ps, wq_sb[:, c, cc * 128:(cc + 1) * 128],
                                         xq[:, c], start=(c == 0), stop=(c == 7))
                    nc.vector.tensor_scalar_add(QT[:, cc, t0:t0 + TBS], ps,
                                                bq_sb[:, cc:cc + 1])

            # ---- attention + output projection, per query block ----
            C = cp.tile([128, 2, S], bf16)   # C^T [cat-chan, tok]
            for qb in range(QB):
                q0 = qb * QBS
                for hp in range(2):
                    c_a = cps.tile([65, QBS], f32, tag="ca", name="c_a")
                    c_b = cps.tile([65, QBS], f32, tag="cb", name="c_b")

                    def pv(kt, e):
                        # C^T accumulation; row 64 = softmax denominators
                        nc.tensor.matmul(c_a, V[:, kt, 130 * hp:130 * hp + 65],
                                         e[:, 0:QBS], start=(kt == 0),
                                         stop=(kt == KT_N - 1))
                        nc.tensor.matmul(c_b, V[:, kt, 130 * hp + 65:130 * hp + 130],
                                         e[:, QBS:2 * QBS], start=(kt == 0),
                                         stop=(kt == KT_N - 1))

                    # kt loop software-pipelined 2 deep: PE runs scores(kt)
                    # while ACT exps kt-1 and PV consumes kt-2.
                    pending = []
                    for kt in range(KT_N):
                        k0 = kt * 128
                        s_ps = sp.tile([128, 2 * QBS], f32, tag="s")
                        # transposed scores for the head pair (row-packed)
                        nc.tensor.matmul(s_ps[:, 0:QBS],
                                         KT[0:64, hp, k0:k0 + 128],
                                         QT[0:64, hp, q0:q0 + QBS])
                        nc.tensor.matmul(s_ps[:, QBS:2 * QBS],
                                         KT[64:128, hp, k0:k0 + 128],
                                         QT[64:128, hp, q0:q0 + QBS])
                        e = ep.tile([128, 2 * QBS], bf16, tag="e")
                        nc.scalar.activation(e, s_ps, AF.Exp, scale=0.125)
                        pending.append((kt, e))
                        if len(pending) > 2:
                            pv(*pending.pop(0))
                    for item in pending:
                        pv(*item)
                    # bv's contribution is a constant vector added on the host
                    s_a = rp.tile([65, QBS], bf16, tag="sa", name="s_a")
                    s_b = rp.tile([65, QBS], bf16, tag="sb", name="s_b")
                    nc.vector.tensor_copy(s_a[64:65, :], c_a[64:65, :])
                    nc.vector.tensor_copy(s_b[64:65, :], c_b[64:65, :])
                    # broadcast sums to 64 partitions, reciprocal on ACT
                    b_ps = pp.tile([64, QBS], f32, tag="pp", name="b_ps")
                    nc.tensor.matmul(b_ps, ones_sb[64:65, :], s_a[64:65, :])
                    r_a = rp.tile([64, QBS], f32, tag="ra", name="r_a")
                    nc.vector.reciprocal_approx_fast(r_a, b_ps)
                    b_ps2 = pp.tile([64, QBS], f32, tag="pp", name="b_ps2")
                    nc.tensor.matmul(b_ps2, ones_sb[64:65, :], s_b[64:65, :])
                    r_b = rp.tile([64, QBS], f32, tag="rb", name="r_b")
                    nc.vector.reciprocal_approx_fast(r_b, b_ps2)
                    nc.vector.tensor_mul(C[0:64, hp, q0:q0 + QBS], c_a[0:64, :], r_a)
                    nc.vector.tensor_mul(C[64:128, hp, q0:q0 + QBS], c_b[0:64, :], r_b)
                # output projection for this query block's tokens
                for tt in range(QBS // 128):
                    tg = q0 + tt * 128
                    o = op.tile([128, D], f32, tag="o")
                    for nb in range(2):
                        ps = pp.tile([128, 512], f32, tag="pp")
                        for cc in range(2):
                            nc.tensor.matmul(ps, C[:, cc, tg:tg + 128],
                                             wo_sb[:, cc, nb * 512:(nb + 1) * 512],
                                             start=(cc == 0), stop=(cc == 1))
                        nc.vector.tensor_copy(o[:, nb * 512:(nb + 1) * 512], ps)
                    nc.sync.dma_start(out[tg:tg + 128, :], o)

    nc.compile()
    return nc


_CACHE = {}


def _get_nc():
    if "nc" not in _CACHE:
        _CACHE["nc"] = build()
    return _CACHE["nc"]


def make_in_maps(q, k, v, wq, bq, wk, bk, wv, bv, wo, bo):
    xT = {}
    for b in range(B):
        xT[("q", b)] = np.ascontiguousarray(np.asarray(q[b]).T).astype(BF)
        xT[("k", b)] = np.ascontiguousarray(np.asarray(k[b]).T).astype(BF)
        xT[("v", b)] = np.ascontiguousarray(np.asarray(v[b]).T).astype(BF)
    in_maps = []
    for core in range(N_CORES):
        b, j = divmod(core, N_CORES // B)
        sl = slice(CL * j, CL * (j + 1))
        in_maps.append({
            "qT": xT[("q", b)],
            "kT": xT[("k", b)],
            "vT": xT[("v", b)],
            "wqT": np.ascontiguousarray(np.asarray(wq)[sl, :].T).astype(BF),
            "wkT": np.ascontiguousarray(np.asarray(wk)[sl, :].T).astype(BF),
            "wvT": np.ascontiguousarray(np.asarray(wv)[sl, :].T).astype(BF),
            "woT": np.ascontiguousarray(np.asarray(wo)[:, sl].T).astype(BF),
            "bq": np.ascontiguousarray(bq[sl], dtype=np.float32),
            "bk": np.ascontiguousarray(bk[sl], dtype=np.float32),
            "vones": np.ones((128, KT_N), dtype=BF),
        })
    return in_maps


def combine(results, bv, wo, bo):
    GP = N_CORES // B
    const = (np.asarray(bv, dtype=np.float64) @ np.asarray(wo, dtype=np.float64).T
             + np.asarray(bo, dtype=np.float64)).astype(np.float32)
    out = np.empty((B, S, D), dtype=np.float32)
    for b in range(B):
        acc = results[b * GP]["out"].astype(np.float32).copy()
        for j in range(1, GP):
            acc += results[b * GP + j]["out"]
        out[b] = acc + const[None, :]
    return out


def kernel(q, k, v, wq, bq, wk, bk, wv, bv, wo, bo):
    nc = _get_nc()
    in_maps = make_in_maps(q, k, v, wq, bq, wk, bk, wv, bv, wo, bo)
    res = bass_utils.run_bass_kernel_spmd(nc, in_maps, core_ids=list(range(N_CORES)))
    return combine(res.results, bv, wo, bo)
